# revision 34
# baseline (speedup 1.0000x reference)
"""Trainium2 Bass kernel for multi-head attention (B=2, S=2048, D=1024, H=16).

Sharding: tensor-parallel over heads x data-parallel over batch. Core c
handles batch b=c//4 and heads 4*(c%4)..4*(c%4)+3 for ALL 2048 query rows.
After the (partial, 4-head) output projection, a 4-core ReduceScatter sums
the head-group partials and leaves core c with query rows [512*(c%4),
512*(c%4+1)) of its batch -- exactly its shard of the final output. The
output bias bo is added on the host after the gather (linear, added once).

Key compaction: pad_mask==0 keys contribute nothing to attention (the
reference gives them -1e9 scores), so the host gathers only the valid key
rows (~1018 of 2048 per batch) into a fixed VMAX=1152 buffer before the
K/V projections. Scores, exp, and attn@V shrink ~1.8x. Padded tail keys
are killed by multiplying V rows (and the softmax-denominator column) by a
0/1 validity vector, exactly like the reference's -1e9 masking.

All matmul operands are bf16 (fp32 PSUM accumulate); measured end-to-end
error vs the fp32 reference is ~7e-3 max-rel (tolerance 2e-2).

Engine balance (from perfetto traces): PE ~106us of matmul rows is the
floor, so everything else hides under it. Exp runs on Act in 3-key-chunk
[128,1536] instructions (PSUM pools are phase-scoped so scores can hold 6
banks during attention). Q biases ride in Act Identity copies; K bias and
the output-projection PSUM drains ride on DVE; the V bias is a K=1
ones-row matmul term; softmax denominators come from a 65th all-ones V
column, are batched per qtile-pair into one [2,512] DVE reciprocal (the
exact reciprocal costs 3.4us per call regardless of partition count),
broadcast via gpsimd, and applied as a DVE multiply one pair late so the
PSUM-freeing copies never queue behind a reciprocal. The ReduceScatter
output DMAs issue from the gpsimd queue after both RS triggers so neither
the Sync DMA queue nor the second collective blocks behind their waits.
"""

import os
import sys

sys.path.insert(0, "/opt/trn_rl_repo")

import numpy as np
import ml_dtypes

B, S, D, H, DH = 2, 2048, 1024, 16, 64
NCORES = 8
GPB = 4                  # head-groups (cores) per batch
HL = H // GPB            # 4 local heads per core
G = HL // 2              # 2 local 2-head groups
P = 128
DCH = D // P             # 8 contraction chunks
VMAX = 1152              # compacted-key capacity (valid ~1018 +- 23)
SCK = VMAX // P          # 9 key chunks
QT = S // 512            # 4 query tiles of 512
OUTR = S // GPB          # 512 output rows owned per core after RS

BF16 = ml_dtypes.bfloat16

_compiled = {}
LAST_RESULTS = None
UNROLL = 1
DEBUG_NOCC = bool(os.environ.get("DEBUG_NOCC"))   # skip RS, emit full partial


def _build_program():
    import concourse.bass as bass  # noqa: F401
    import concourse.mybir as mybir
    import concourse.tile as tile
    from concourse import bacc

    f32 = mybir.dt.float32
    bf16 = mybir.dt.bfloat16
    AF = mybir.ActivationFunctionType
    OP = mybir.AluOpType

    nc = bacc.Bacc(
        "TRN2", target_bir_lowering=False, debug=False,
        num_devices=NCORES,
    )

    xT = nc.dram_tensor("xT", [DCH, P, S], bf16, kind="ExternalInput")
    xkT = nc.dram_tensor("xkT", [DCH, P, VMAX], bf16, kind="ExternalInput")
    wq = nc.dram_tensor("wq", [G, P, DCH, P], bf16, kind="ExternalInput")
    wk = nc.dram_tensor("wk", [G, P, DCH, P], bf16, kind="ExternalInput")
    wv = nc.dram_tensor("wv", [P, DCH, 2 * P], bf16, kind="ExternalInput")
    woT = nc.dram_tensor("woT", [G, P, D], bf16, kind="ExternalInput")
    bq = nc.dram_tensor("bq", [P, G], f32, kind="ExternalInput")
    bk = nc.dram_tensor("bk", [P, G], f32, kind="ExternalInput")
    bv = nc.dram_tensor("bv", [1, 2 * P], bf16, kind="ExternalInput")
    maskT = nc.dram_tensor("maskT", [P, SCK], f32, kind="ExternalInput")
    out = nc.dram_tensor(
        "out", [S if DEBUG_NOCC else OUTR, D], bf16, kind="ExternalOutput")

    RG = [[0, 1, 2, 3], [4, 5, 6, 7]]

    with tile.TileContext(nc) as tc:
        with (
            tc.tile_pool(name="const", bufs=1) as constp,
            tc.tile_pool(name="xq", bufs=DCH) as xqp,
            tc.tile_pool(name="xk", bufs=DCH) as xkp,
            tc.tile_pool(name="w", bufs=1) as wpool,
            tc.tile_pool(name="kt", bufs=G) as ktpool,
            tc.tile_pool(name="va", bufs=SCK) as vpool,
            tc.tile_pool(name="qz", bufs=1) as qzpool,
            tc.tile_pool(name="pt", bufs=6) as ptpool,
            tc.tile_pool(name="cat", bufs=2) as catp,
            tc.tile_pool(name="rr", bufs=6) as rpool,
            tc.tile_pool(name="dpo", bufs=4, space="DRAM") as dpop,
            tc.tile_pool(name="drs", bufs=4, space="DRAM") as drsp,
        ):
            # ---- constants
            bq_sb = constp.tile([P, G], f32, tag="bq")
            nc.sync.dma_start(out=bq_sb[:], in_=bq[:])
            bk_sb = constp.tile([P, G], f32, tag="bk")
            nc.sync.dma_start(out=bk_sb[:], in_=bk[:])
            bv_sb = constp.tile([1, 2 * P], bf16, tag="bv")
            nc.sync.dma_start(out=bv_sb[:], in_=bv[:])
            mask_sb = constp.tile([P, SCK], f32, tag="mask")
            nc.sync.dma_start(out=mask_sb[:], in_=maskT[:])
            ones_sb = constp.tile([1, 512], bf16, tag="ones")
            nc.vector.memset(ones_sb[:], 1.0)
            qzp = []
            for hl in range(HL):
                par = hl % 2
                qz = qzpool.tile([P, S], bf16, tag=f"qz{hl}",
                                 name=f"qz_{hl}")
                nc.vector.memset(
                    qz[(1 - par) * 64:(2 - par) * 64, :], 0.0)
                qzp.append(qz)

            for rep in range(UNROLL):
                concat = catp.tile([P, G, S], bf16, tag="cat",
                                   name=f"cat{rep}")

                # loads ordered so the K projection can start earliest
                wk_sb, wq_sb = [], []
                for g in range(G):
                    t = wpool.tile([P, DCH, P], bf16, tag=f"wk{g}")
                    nc.sync.dma_start(out=t[:], in_=wk[g])
                    wk_sb.append(t)
                xk_t = []
                for d in range(DCH):
                    t = xkp.tile([P, VMAX], bf16, tag="xk",
                                 name=f"xk{rep}_{d}")
                    nc.sync.dma_start(out=t[:], in_=xkT[d])
                    xk_t.append(t)
                wv_sb = wpool.tile([P, DCH, 2 * P], bf16, tag="wv")
                nc.sync.dma_start(out=wv_sb[:], in_=wv[:])
                for g in range(G):
                    t = wpool.tile([P, DCH, P], bf16, tag=f"wq{g}")
                    nc.sync.dma_start(out=t[:], in_=wq[g])
                    wq_sb.append(t)
                xq_t = []
                for d in range(DCH):
                    t = xqp.tile([P, S], bf16, tag="xq", name=f"xq{rep}_{d}")
                    nc.sync.dma_start(out=t[:], in_=xT[d])
                    xq_t.append(t)
                wo_sb = []
                for g in range(G):
                    t = wpool.tile([P, D], bf16, tag=f"wo{g}")
                    nc.sync.dma_start(out=t[:], in_=woT[g])
                    wo_sb.append(t)

                # ---- projections (own PSUM scope: 2 banks)
                kt = []
                with tc.tile_pool(name=f"pp{rep}", bufs=2,
                                  space="PSUM") as pp:
                    # K projection: kt[g] = [128 (par*64+dh), VMAX] bf16
                    KCH = [(0, 512), (512, 512), (1024, VMAX - 1024)]
                    for g in range(G):
                        ktile = ktpool.tile([P, VMAX], bf16, tag="kt")
                        for off, w in KCH:
                            ps = pp.tile([P, 512], f32, tag="pp")
                            for d in range(DCH):
                                nc.tensor.matmul(
                                    ps[:, 0:w],
                                    wk_sb[g][:, d, :],
                                    xk_t[d][:, off:off + w],
                                    start=(d == 0),
                                    stop=(d == DCH - 1),
                                )
                            nc.vector.tensor_scalar_add(
                                ktile[:, off:off + w], ps[:, 0:w],
                                bk_sb[:, g:g + 1],
                            )
                        kt.append(ktile)

                    # V projection: va[sc] = [128 s, 4 heads, 64+1] bf16
                    va = []
                    for sc in range(SCK):
                        vt = vpool.tile([P, HL, 65], bf16, tag="va")
                        ps = pp.tile([P, 512], f32, tag="pp",
                                     name=f"vps_{rep}_{sc}")[:, 0:256]
                        nc.tensor.matmul(
                            ps[:],
                            ones_sb[0:1, 0:P],
                            bv_sb[0:1, :],
                            start=True, stop=False,
                        )
                        for d in range(DCH):
                            nc.tensor.matmul(
                                ps[:],
                                xk_t[d][:, sc * P:(sc + 1) * P],
                                wv_sb[:, d, :],
                                start=False,
                                stop=(d == DCH - 1),
                            )
                        ps_r = ps.rearrange("p (h e) -> p h e", e=64)
                        nc.scalar.activation(
                            vt[:, :, 0:64], ps_r, AF.Copy,
                            bias=0.0, scale=mask_sb[:, sc:sc + 1],
                        )
                        nc.vector.tensor_scalar(
                            vt[:, :, 64:65], ps_r[:, :, 0:1], 0.0,
                            mask_sb[:, sc:sc + 1], OP.mult, OP.add,
                        )
                        va.append(vt)

                    # Q projection into per-head zero-padded tiles
                    # (tiles pre-zeroed once before the body loop)
                    for g in range(G):
                        for qt in range(QT):
                            ps = pp.tile([P, 512], f32, tag="pp")
                            for d in range(DCH):
                                nc.tensor.matmul(
                                    ps[:],
                                    wq_sb[g][:, d, :],
                                    xq_t[d][:, qt * 512:(qt + 1) * 512],
                                    start=(d == 0),
                                    stop=(d == DCH - 1),
                                )
                            for par in range(2):
                                lo, hi = par * 64, (par + 1) * 64
                                nc.vector.tensor_scalar_add(
                                    qzp[2 * g + par][lo:hi,
                                                     qt * 512:(qt + 1) * 512],
                                    ps[lo:hi, :],
                                    bq_sb[lo:hi, g:g + 1],
                                )

                # ---- attention (PSUM: scores 3banks x2 + out 1bank x2)
                GRPS = [(0, 3), (3, 3), (6, 3)]
                with (
                    tc.tile_pool(name=f"psc{rep}", bufs=2,
                                 space="PSUM") as psc,
                    tc.tile_pool(name=f"po{rep}", bufs=2,
                                 space="PSUM") as pop,
                ):
                    # flat software pipeline over all (head, qtile, group)
                    # score-groups: PE never drains at (head,qtile)
                    # boundaries waiting on exp; attn@V PSUM is copied to
                    # SBUF immediately so the 2-deep PSUM ring never blocks
                    # on the normalization chain.
                    NG = len(GRPS)
                    iters = [(2 * g + par, qt)
                             for g in range(G)
                             for par in range(2)
                             for qt in range(QT)]
                    stream = [(it, gi)
                              for it in range(len(iters))
                              for gi in range(NG)]
                    po_tile = {}
                    pt_tiles = {}
                    pend = {}

                    def emit_scores(it, gi):
                        hl, qt = iters[it]
                        g = hl // 2
                        qsl = slice(qt * 512, (qt + 1) * 512)
                        sc0, n = GRPS[gi]
                        sps = psc.tile([P, 3, 512], f32, tag="ps", name=f"sps_{rep}_{it}_{gi}")
                        for j in range(n):
                            sc = sc0 + j
                            nc.tensor.matmul(
                                sps[:, j, :],
                                kt[g][:, sc * P:(sc + 1) * P],
                                qzp[hl][:, qsl],
                                start=True, stop=True,
                            )
                        pt = ptpool.tile([P, 3, 512], bf16, tag="pt", name=f"pt_{rep}_{it}_{gi}")
                        nc.scalar.activation(
                            pt[:, 0:n, :], sps[:, 0:n, :],
                            AF.Exp, bias=0.0, scale=0.125,
                        )
                        pt_tiles[(it, gi)] = pt

                    pending_norm = []

                    def emit_norm(it, hl, qt, den2, sbs):
                        """Reciprocal + broadcast + multiply for one pair of
                        qtiles. Multiplies run on gpsimd (SBUF->SBUF) so the
                        DVE queue stays free for the PSUM-draining copies."""
                        den2r = rpool.tile([2, 512], f32, tag="den2r", name=f"den2r_{rep}_{it}")
                        nc.vector.reciprocal(den2r[:], den2[:])
                        g, par = hl // 2, hl % 2
                        for qq in (qt - 1, qt):
                            qsl = slice(qq * 512, (qq + 1) * 512)
                            po_sb = sbs[qq]
                            d0 = rpool.tile([1, 512], f32, tag="d0", name=f"d0_{rep}_{it}_{qq}")
                            nc.sync.dma_start(
                                out=d0[:],
                                in_=den2r[qq % 2:qq % 2 + 1, :])
                            rep_t = rpool.tile([64, 512], f32, tag="rep", name=f"rept_{rep}_{it}_{qq}")
                            nc.gpsimd.partition_broadcast(
                                rep_t[:], d0[0:1, :], channels=64)
                            if par == 0:
                                nc.vector.tensor_tensor(
                                    concat[0:64, g, qsl], po_sb[0:64, :],
                                    rep_t[:], OP.mult,
                                )
                            else:
                                tmp = rpool.tile([64, 512], bf16, tag="tmp", name=f"tmp_{rep}_{it}_{qq}")
                                nc.vector.tensor_tensor(
                                    tmp[:], po_sb[0:64, :], rep_t[:],
                                    OP.mult,
                                )
                                nc.sync.dma_start(
                                    out=concat[64:P, g, qsl], in_=tmp[:],
                                )

                    def finish_iter(it):
                        """po -> SBUF, stash den; queue the pair's normalize
                        one pair late so the DVE copies that free the attnV
                        PSUM ring never wait behind a reciprocal."""
                        hl, qt = iters[it]
                        po_t = po_tile.pop(it)
                        po_sb = rpool.tile([65, 512], f32, tag="posb", name=f"posb_{rep}_{it}")
                        nc.vector.tensor_copy(po_sb[:], po_t[0:65, :])
                        ql = qt % 2
                        if ql == 0:
                            den2 = rpool.tile([2, 512], f32, tag="den2", name=f"den2_{rep}_{it}")
                            pend[hl] = (den2, {})
                        den2, sbs = pend[hl]
                        nc.sync.dma_start(
                            out=den2[ql:ql + 1, :], in_=po_sb[64:65, :])
                        sbs[qt] = po_sb
                        if ql != 1:
                            return
                        pending_norm.append((it, hl, qt, den2, sbs))
                        if len(pending_norm) > 1:
                            emit_norm(*pending_norm.pop(0))

                    def emit_o(it, gi):
                        hl, qt = iters[it]
                        sc0, n = GRPS[gi]
                        if gi == 0:
                            po_tile[it] = pop.tile([P, 512], f32, tag="po", name=f"po_{rep}_{it}")
                        po_t = po_tile[it]
                        pt = pt_tiles.pop((it, gi))
                        for j in range(n):
                            sc = sc0 + j
                            nc.tensor.matmul(
                                po_t[0:65, :],
                                va[sc][:, hl, :],
                                pt[:, j, :],
                                start=(sc == 0),
                                stop=(sc == SCK - 1),
                            )
                        if gi == NG - 1:
                            finish_iter(it)

                    LOOKAHEAD = 2
                    for idx, (it, gi) in enumerate(stream):
                        emit_scores(it, gi)
                        if idx >= LOOKAHEAD:
                            emit_o(*stream[idx - LOOKAHEAD])
                    for idx in range(len(stream) - LOOKAHEAD,
                                     len(stream)):
                        emit_o(*stream[idx])
                    while pending_norm:
                        emit_norm(*pending_norm.pop(0))

                # ---- output projection (partial over this core's 256
                # concat dims) -> bf16 -> DRAM -> ReduceScatter per
                # 512-feature half.
                with tc.tile_pool(name=f"pp2{rep}", bufs=2,
                                  space="PSUM") as pp2:
                    rs_ts = []
                    for eb in range(2):
                        poF = dpop.tile([S, 512], bf16, tag="pof",
                                        name=f"poF{rep}_{eb}")
                        for qi in range(S // P):
                            ps = pp2.tile([P, 512], f32, tag="pp2")
                            for g in range(G):
                                nc.tensor.matmul(
                                    ps[:],
                                    concat[:, g, qi * P:(qi + 1) * P],
                                    wo_sb[g][:, eb * 512:(eb + 1) * 512],
                                    start=(g == 0),
                                    stop=(g == G - 1),
                                )
                            osb = rpool.tile([P, 512], bf16, tag="osb")
                            if qi % 2 == 0:
                                nc.vector.tensor_copy(osb[:], ps[:])
                            else:
                                nc.scalar.activation(
                                    osb[:], ps[:], AF.Copy,
                                    bias=0.0, scale=1.0)
                            if DEBUG_NOCC:
                                nc.sync.dma_start(
                                    out=out[qi * P:(qi + 1) * P,
                                            eb * 512:(eb + 1) * 512],
                                    in_=osb[:],
                                )
                                continue
                            nc.sync.dma_start(
                                out=poF[qi * P:(qi + 1) * P, :], in_=osb[:],
                            )
                        if DEBUG_NOCC:
                            continue
                        rs_t = drsp.tile([OUTR, 512], bf16, tag="rs",
                                         name=f"rs{rep}_{eb}")
                        nc.gpsimd.collective_compute(
                            "ReduceScatter",
                            mybir.AluOpType.add,
                            replica_groups=RG,
                            ins=[poF[:].opt()],
                            outs=[rs_t[:].opt()],
                        )
                        rs_ts.append(rs_t)
                    # output DMAs wait on the collectives; issue them from
                    # the gpsimd queue AFTER both RS triggers so neither the
                    # Sync DMA queue (next body's small latency-critical
                    # DMAs) nor the second RS is blocked behind the waits.
                    for eb, rs_t in enumerate(rs_ts):
                        nc.gpsimd.dma_start(
                            out=out[:, eb * 512:(eb + 1) * 512], in_=rs_t[:],
                        )

    nc.compile()
    nc.finalize()
    return nc


def _np_fallback(x, pad_mask, wq, wk, wv, bq, bk, bv, wo, bo):
    """Reference math in numpy; only for absurd masks (valid keys > VMAX)."""
    q = np.einsum('bsd,hdk->bhsk', x, wq) + bq[None, :, None, :]
    k = np.einsum('bsd,hdk->bhsk', x, wk) + bk[None, :, None, :]
    v = np.einsum('bsd,hdk->bhsk', x, wv) + bv[None, :, None, :]
    s = np.einsum('bhqk,bhsk->bhqs', q, k) / np.sqrt(np.float32(DH))
    s = np.where(pad_mask[:, None, None, :] != 0, s, -1e9)
    s -= s.max(axis=-1, keepdims=True)
    p = np.exp(s)
    p /= p.sum(axis=-1, keepdims=True)
    h = np.einsum('bhqs,bhsk->bhqk', p, v)
    cat = h.transpose(0, 2, 1, 3).reshape(B, S, D)
    return (cat @ wo.T + bo).astype(np.float32)


def prep_inputs(x, pad_mask, wq, wk, wv, bq, bk, bv, wo, bo):
    """Build per-core input maps (host-side shard + layout prep)."""
    x = np.ascontiguousarray(np.asarray(x, dtype=np.float32))
    pad_mask = np.asarray(pad_mask)
    wq = np.asarray(wq, dtype=np.float32)
    wk = np.asarray(wk, dtype=np.float32)
    wv = np.asarray(wv, dtype=np.float32)
    bq = np.asarray(bq, dtype=np.float32)
    bk = np.asarray(bk, dtype=np.float32)
    bv = np.asarray(bv, dtype=np.float32)
    wo = np.asarray(wo, dtype=np.float32)

    def bf(a):
        return np.ascontiguousarray(a).astype(BF16)

    # per-batch compacted keys + validity
    xT_b, xkT_b, mask_b = [], [], []
    for b in range(B):
        idx = np.nonzero(pad_mask[b])[0]
        nv = len(idx)
        if nv > VMAX:
            return None
        xT_b.append(bf(x[b].T).reshape(DCH, P, S))
        xk = np.zeros((VMAX, D), np.float32)
        xk[:nv] = x[b][idx]
        xkT_b.append(bf(xk.T).reshape(DCH, P, VMAX))
        m = np.zeros(VMAX, np.float32)
        m[:nv] = 1.0
        mask_b.append(np.ascontiguousarray(m.reshape(SCK, P).T))

    in_maps = []
    for c in range(NCORES):
        b, hg = c // GPB, c % GPB
        hs = slice(HL * hg, HL * (hg + 1))

        def packed2(w):
            ws = w[hs]  # [4, D, 64]
            arr = np.empty((G, P, DCH, P), np.float32)
            for g in range(G):
                m = ws[2 * g:2 * g + 2].transpose(1, 0, 2).reshape(D, P)
                arr[g] = m.reshape(DCH, P, P).transpose(1, 0, 2)
            return bf(arr)

        wv_m = wv[hs].transpose(1, 0, 2).reshape(D, 2 * P)
        wv_dev = bf(wv_m.reshape(DCH, P, 2 * P).transpose(1, 0, 2))
        woT_dev = np.empty((G, P, D), np.float32)
        for g in range(G):
            st = 256 * hg + 128 * g
            woT_dev[g] = wo[:, st:st + P].T
        in_maps.append({
            "xT": xT_b[b], "xkT": xkT_b[b],
            "wq": packed2(wq), "wk": packed2(wk), "wv": wv_dev,
            "woT": bf(woT_dev),
            "bq": np.ascontiguousarray(bq[hs].reshape(G, P).T),
            "bk": np.ascontiguousarray(bk[hs].reshape(G, P).T),
            "bv": bf(bv[hs].reshape(1, 2 * P)),
            "maskT": mask_b[b],
        })
    return in_maps


def kernel(**inputs):
    global LAST_RESULTS
    from concourse.bass_utils import run_bass_kernel_spmd

    in_maps = prep_inputs(**inputs)
    if in_maps is None:
        return _np_fallback(**{k: np.asarray(v, dtype=np.float32)
                               if k != "pad_mask" else np.asarray(v)
                               for k, v in inputs.items()})

    if "nc" not in _compiled:
        _compiled["nc"] = _build_program()
    nc = _compiled["nc"]

    res = run_bass_kernel_spmd(
        nc, in_maps, list(range(NCORES)),
        trace=bool(os.environ.get("BASS_TRACE")),
    )
    LAST_RESULTS = res

    bo = np.asarray(inputs["bo"], dtype=np.float32)
    out = np.empty((B, S, D), dtype=np.float32)
    for c in range(NCORES):
        b, hg = c // GPB, c % GPB
        out[b, hg * OUTR:(hg + 1) * OUTR, :] = (
            res.results[c]["out"].astype(np.float32) + bo)
    return out
